# revision 1
# baseline (speedup 1.0000x reference)
"""Trainium2 Bass kernel for nn_CTRPredictor (gnn_message_passing).

score[e] = dot(normalize(x[src[e]]), normalize(x[dst[e]]))  for E edges.

Strategy (8 NeuronCores, SPMD):
  - Edges sharded: core i gets edges [i*80000, (i+1)*80000).
  - Each core L2-normalizes its 12500-node slice of x (ACT square, DVE
    reduce, sqrt, reciprocal, scale) to bf16; two half AllGathers replicate
    the normalized table to every core as 4 banks of 25000 rows (gathers on
    the first two banks overlap the second collective).
  - Host groups each core's edges by (src_bank, dst_bank) into 16 groups
    (so bank-local indices fit dma_gather's int16) with a fixed padded
    capacity per group (pad slots gather row 0 and are discarded).
  - Per group: dma_gather x_norm[src] and x_norm[dst] rows (256B bf16)
    across 4 SWDGE queues, DVE bf16 multiply + grouped reduce -> scores.
  - Host un-permutes scores back to edge order.
"""

import numpy as np

N = 100000
D = 128
E = 640000
CORES = 8
EPC = E // CORES          # 80000 edges per core
SLICE = N // CORES        # 12500 nodes normalized per core
QSL = SLICE // 4          # 3125-row quarter slices (AllGather chunks)
NBANK = 4
BANK = N // NBANK         # 25000 rows per stripe bank
NGRP = NBANK * NBANK      # 16 (src_bank, dst_bank) groups
GCAP = 5376               # padded edge capacity per group (42*128)
NCALLG = 2                # gather calls per group per endpoint
GCALL = GCAP // NCALLG    # 2688 indices per dma_gather call
CCOL = GCALL // 128       # 21 gathered row-columns per call
ICOL = GCALL // 16        # 168 index columns per call
NCALL = NGRP * NCALLG     # 32 slot-range calls (each does src + dst)
SCOL = NGRP * GCAP // 128  # 672 score columns
SP_NORM = 100             # partitions used in the normalize phase
RN = SLICE // SP_NORM     # 125 rows per partition in normalize phase

_CACHE = {}
LAST_RESULTS = None
RUN_KWARGS = {}  # extra kwargs for run_bass_kernel_spmd (used by test harness)


def _build():
    from concourse import bass, bacc, tile, mybir

    f32 = mybir.dt.float32
    bf16 = mybir.dt.bfloat16
    i16 = mybir.dt.int16
    i32 = mybir.dt.int32

    nc = bacc.Bacc("TRN2", target_bir_lowering=False, debug=False,
                   num_devices=CORES, num_swdge_queues=4,
                   dynamic_dma_scratch_size=40960)

    xsl_d = nc.dram_tensor("xsl", [SP_NORM, RN * D], f32, kind="ExternalInput")
    sidx_d = nc.dram_tensor("src_idx", [128, NCALL * ICOL], i16,
                            kind="ExternalInput")
    didx_d = nc.dram_tensor("dst_idx", [128, NCALL * ICOL], i16,
                            kind="ExternalInput")
    out_d = nc.dram_tensor("out", [128, SCOL], f32, kind="ExternalOutput")

    with tile.TileContext(nc) as tc:
        with tc.tile_pool(name="dram", bufs=1, space="DRAM") as dp, \
             tc.tile_pool(name="persist", bufs=1) as pp:

            # ---- index tables + score accumulator ----
            sidx = pp.tile([128, NCALL * ICOL], i16)
            didx = pp.tile([128, NCALL * ICOL], i16)
            nc.sync.dma_start(out=sidx[:, :], in_=sidx_d.ap())
            nc.sync.dma_start(out=didx[:, :], in_=didx_d.ap())
            score = pp.tile([128, SCOL], f32)

            # ---- phase 0: normalize this core's slice to bf16 ----
            banks = []
            with tc.tile_pool(name="ph0", bufs=1) as p0, \
                 tc.tile_pool(name="sqp", bufs=2) as sqp:
                xsl = p0.tile([SP_NORM, RN * D], f32)
                nc.sync.dma_start(out=xsl[:, :], in_=xsl_d.ap())
                ns = p0.tile([SP_NORM, RN], f32)
                rchunk = RN // 5
                for rc in range(5):
                    sq = sqp.tile([SP_NORM, rchunk * D], f32, tag="sq")
                    nc.scalar.activation(
                        out=sq[:, :],
                        in_=xsl[:, rc * rchunk * D:(rc + 1) * rchunk * D],
                        func=mybir.ActivationFunctionType.Square)
                    nc.vector.tensor_reduce(
                        out=ns[:, rc * rchunk:(rc + 1) * rchunk],
                        in_=sq[:, :].rearrange("p (r d) -> p r d", d=D),
                        axis=mybir.AxisListType.X,
                        op=mybir.AluOpType.add,
                    )
                nrm = p0.tile([SP_NORM, RN], f32)
                nc.scalar.activation(out=nrm[:, :], in_=ns[:, :],
                                     func=mybir.ActivationFunctionType.Sqrt)
                rns = p0.tile([SP_NORM, RN], f32)
                nc.vector.reciprocal(out=rns[:, :], in_=nrm[:, :])
                ntile = p0.tile([SP_NORM, RN * D], bf16)
                nc.vector.tensor_mul(
                    out=ntile[:, :].rearrange("p (r d) -> p r d", d=D),
                    in0=xsl[:, :].rearrange("p (r d) -> p r d", d=D),
                    in1=rns[:, :].unsqueeze(-1).to_broadcast(
                        [SP_NORM, RN, D]),
                )

                # ---- two half AllGathers of the normalized slice ----
                # Half h gathers rows [h*6250, (h+1)*6250) of every core's
                # slice; with the stripe node map below, half h's output is
                # banks 2h and 2h+1 (gathers on early banks overlap the
                # second collective).
                for h in range(2):
                    agin = dp.tile([SLICE // 2, D], bf16, name=f"agin{h}")
                    htab = dp.tile([N // 2, D], bf16, name=f"htab{h}",
                                   addr_space="Shared")
                    nc.sync.dma_start(
                        out=agin[:, :].rearrange("(p r) d -> p (r d)", p=50),
                        in_=ntile[50 * h:50 * (h + 1), :],
                    )
                    nc.gpsimd.collective_compute(
                        "AllGather",
                        mybir.AluOpType.bypass,
                        replica_groups=[list(range(CORES))],
                        ins=[agin.opt()],
                        outs=[htab.opt()],
                    )
                    banks.append(htab[:BANK, :])
                    banks.append(htab[BANK:, :])

            # ---- main loop: gathers on 4 queues, DVE dot per call ----
            # process groups in bank-availability order: a group needs banks
            # (a, b), and AllGather c completes before c+1 — order by max
            group_order = sorted(range(NGRP),
                                 key=lambda g: (max(g // NBANK, g % NBANK),
                                                g // NBANK, g % NBANK))
            with tc.tile_pool(name="ga", bufs=5) as ga, \
                 tc.tile_pool(name="gb", bufs=5) as gb:
                qn = 0
                for g in group_order:
                    ba, bb = g // NBANK, g % NBANK
                    for c in range(NCALLG):
                        call = g * NCALLG + c
                        col0 = call * ICOL
                        xs_t = ga.tile([128, CCOL * D], bf16, tag="A")
                        xd_t = gb.tile([128, CCOL * D], bf16, tag="B")
                        nc.gpsimd.dma_gather(
                            out_ap=xs_t[:, :].rearrange(
                                "p (c d) -> p c d", d=D),
                            in_ap=banks[ba][:, :],
                            idxs_ap=sidx[:, col0:col0 + ICOL],
                            num_idxs=GCALL, num_idxs_reg=GCALL, elem_size=D,
                            single_packet=False, queue_num=qn % 4,
                        )
                        qn += 1
                        nc.gpsimd.dma_gather(
                            out_ap=xd_t[:, :].rearrange(
                                "p (c d) -> p c d", d=D),
                            in_ap=banks[bb][:, :],
                            idxs_ap=didx[:, col0:col0 + ICOL],
                            num_idxs=GCALL, num_idxs_reg=GCALL, elem_size=D,
                            single_packet=False, queue_num=qn % 4,
                        )
                        qn += 1
                        nc.vector.tensor_mul(out=xs_t[:, :], in0=xs_t[:, :],
                                             in1=xd_t[:, :])
                        sc0 = call * CCOL
                        nc.vector.tensor_reduce(
                            out=score[:, sc0:sc0 + CCOL],
                            in_=xs_t[:, :].rearrange("p (c d) -> p c d", d=D),
                            axis=mybir.AxisListType.X,
                            op=mybir.AluOpType.add,
                        )

                nc.sync.dma_start(out=out_d.ap(), in_=score[:, :])

    nc.compile()
    return nc


def _node_map(n):
    """node id -> (bank, bank-local index) for the half-AllGather layout.

    Half h of core r's slice lands at rows [r*6250, r*6250+6250) of htab_h;
    banks 2h / 2h+1 are htab_h's two 25000-row halves.
    """
    r = n // SLICE
    rem = n - r * SLICE
    h = rem // (SLICE // 2)
    rem2 = rem - h * (SLICE // 2)
    bank = 2 * h + r // 4
    local = (r % 4) * (SLICE // 2) + rem2
    return bank, local


def _wrap_idx(flat):
    """[GCALL] int16 -> [128, ICOL] in dma_gather's 16-partition wrap."""
    blk = flat.reshape(ICOL, 16).T  # index i at [i%16, i//16]
    return np.tile(blk, (8, 1))


def _prepare_core(src_l, dst_l):
    """Group one core's edges by bank pair; build index tilings + inverse."""
    sb, sl = _node_map(src_l)
    db, dl = _node_map(dst_l)
    key = sb * NBANK + db
    order = np.argsort(key, kind="stable")
    sizes = np.bincount(key, minlength=NGRP)
    if sizes.max() > GCAP:
        raise ValueError(f"group overflow: {sizes.max()} > {GCAP}")
    if sizes.min() <= GCALL:
        raise ValueError(f"group underflow: {sizes.min()} <= {GCALL}")

    sidx = np.zeros((128, NCALL * ICOL), dtype=np.int16)
    didx = np.zeros((128, NCALL * ICOL), dtype=np.int16)
    # inverse: score of edge order[...] lives at [row, col] of out tile
    rows = np.empty(EPC, dtype=np.int64)
    cols = np.empty(EPC, dtype=np.int64)
    off = 0
    for g in range(NGRP):
        ids = order[off:off + sizes[g]]
        off += sizes[g]
        # ascending src addresses give the src-side gather descriptors
        # HBM locality (the dst side stays random)
        ids = ids[np.argsort(sl[ids], kind="stable")]
        s_pad = np.zeros(GCAP, dtype=np.int16)
        d_pad = np.zeros(GCAP, dtype=np.int16)
        s_pad[:ids.size] = sl[ids]
        d_pad[:ids.size] = dl[ids]
        for c in range(NCALLG):
            call = g * NCALLG + c
            col0 = call * ICOL
            seg = slice(c * GCALL, (c + 1) * GCALL)
            sidx[:, col0:col0 + ICOL] = _wrap_idx(s_pad[seg])
            didx[:, col0:col0 + ICOL] = _wrap_idx(d_pad[seg])
        j = np.arange(ids.size)
        rows[ids] = j % 128
        cols[ids] = g * (GCAP // 128) + j // 128
    return sidx, didx, rows, cols


def kernel(x, src, dst):
    global LAST_RESULTS
    from concourse.bass_utils import run_bass_kernel_spmd

    if "nc" not in _CACHE:
        _CACHE["nc"] = _build()
    nc = _CACHE["nc"]

    x32 = np.ascontiguousarray(np.asarray(x, dtype=np.float32))
    src_i = np.asarray(src).astype(np.int64)
    dst_i = np.asarray(dst).astype(np.int64)

    in_maps = []
    inv = []
    for i in range(CORES):
        sidx, didx, rows, cols = _prepare_core(
            src_i[i * EPC:(i + 1) * EPC], dst_i[i * EPC:(i + 1) * EPC])
        inv.append((rows, cols))
        in_maps.append({
            "xsl": np.ascontiguousarray(
                x32[i * SLICE:(i + 1) * SLICE].reshape(SP_NORM, RN * D)),
            "src_idx": np.ascontiguousarray(sidx),
            "dst_idx": np.ascontiguousarray(didx),
        })

    res = run_bass_kernel_spmd(nc, in_maps, core_ids=list(range(CORES)),
                               **RUN_KWARGS)
    LAST_RESULTS = res

    out = np.empty(E, dtype=np.float32)
    for i in range(CORES):
        tilev = np.asarray(res.results[i]["out"])
        rows, cols = inv[i]
        out[i * EPC:(i + 1) * EPC] = tilev[rows, cols]
    return out.reshape(E, 1)



# revision 2
# speedup vs baseline: 1.2192x; 1.2192x over previous
"""Trainium2 Bass kernel for nn_CTRPredictor (gnn_message_passing).

score[e] = dot(normalize(x[src[e]]), normalize(x[dst[e]]))  for E edges.

Strategy (8 NeuronCores, SPMD), v2:
  - Edges sharded: core i gets edges [i*80000, (i+1)*80000).
  - Each core L2-normalizes its 12500-node slice of x in 4 pipelined
    quarter-chunks ([125, 100*128] layout so每 chunk uses 125 partitions),
    emitting each normalized quarter to a quarter AllGather as soon as it
    is ready; AG chunk q produces table bank q (25000 rows, bf16) so
    gathers on early banks overlap later collectives.
  - Host groups each core's edges by (src_bank, dst_bank) into 16 groups
    (bank-local indices fit dma_gather's int16) with JIT-exact capacities
    (max over cores, rounded to 128) instead of a fixed padded capacity.
  - Per group: dma_gather x_norm[src] and x_norm[dst] rows (256B bf16)
    across 4 SWDGE queues, DVE bf16 multiply + grouped reduce -> scores.
  - Host un-permutes scores back to edge order.
"""

import numpy as np

N = 100000
D = 128
E = 640000
CORES = 8
EPC = E // CORES          # 80000 edges per core
SLICE = N // CORES        # 12500 nodes normalized per core
QSL = SLICE // 4          # 3125-row quarter slices (AllGather chunks)
NBANK = 4
BANK = N // NBANK         # 25000 rows per bank (= one AG chunk output)
NGRP = NBANK * NBANK      # 16 (src_bank, dst_bank) groups
GCALL = 2688              # max indices per dma_gather call

# groups in bank-availability order: group (a,b) is gatherable once
# AllGather max(a,b) has completed
GROUP_ORDER = sorted(range(NGRP),
                     key=lambda g: (max(g // NBANK, g % NBANK),
                                    g // NBANK, g % NBANK))

_CACHE = {}
LAST_RESULTS = None
RUN_KWARGS = {}  # extra kwargs for run_bass_kernel_spmd (used by test harness)


def _call_caps(cap):
    """Split a group capacity (multiple of 128) into dma_gather call sizes."""
    calls = []
    while cap > 0:
        c = min(GCALL, cap)
        calls.append(c)
        cap -= c
    return calls


def _build(caps):
    """caps: tuple of 16 per-group capacities (each a multiple of 128)."""
    from concourse import bass, bacc, tile, mybir

    f32 = mybir.dt.float32
    bf16 = mybir.dt.bfloat16
    i16 = mybir.dt.int16

    icols_total = sum(caps) // 16
    scol_total = sum(caps) // 128

    nc = bacc.Bacc("TRN2", target_bir_lowering=False, debug=False,
                   num_devices=CORES, num_swdge_queues=4,
                   dynamic_dma_scratch_size=40960)

    # node slice in [125, 100*128] layout: slice-local node n with
    # q=n//3125, w=n%3125 sits at partition w//25, col q*25 + w%25.
    xsl_d = nc.dram_tensor("xsl", [125, 100 * D], f32, kind="ExternalInput")
    sidx_d = nc.dram_tensor("src_idx", [128, icols_total], i16,
                            kind="ExternalInput")
    didx_d = nc.dram_tensor("dst_idx", [128, icols_total], i16,
                            kind="ExternalInput")
    out_d = nc.dram_tensor("out", [128, scol_total], f32,
                           kind="ExternalOutput")

    CCH = 25 * D  # free-dim span of one quarter chunk [125, 25*128]

    with tile.TileContext(nc) as tc:
        with tc.tile_pool(name="dram", bufs=1, space="DRAM") as dp, \
             tc.tile_pool(name="persist", bufs=1) as pp:

            # ---- index tables + score accumulator ----
            sidx = pp.tile([128, icols_total], i16)
            didx = pp.tile([128, icols_total], i16)
            nc.sync.dma_start(out=sidx[:, :], in_=sidx_d.ap())
            nc.sync.dma_start(out=didx[:, :], in_=didx_d.ap())
            score = pp.tile([128, scol_total], f32)

            # ---- phase 0: normalize this core's slice in 4 chunks ----
            banks = []
            with tc.tile_pool(name="ph0", bufs=1) as p0, \
                 tc.tile_pool(name="sqp", bufs=2) as sqp:
                xsl = p0.tile([125, 100 * D], f32)
                ntile = p0.tile([125, 100 * D], bf16)
                ns = p0.tile([125, 100], f32)
                nrm = p0.tile([125, 100], f32)
                rns = p0.tile([125, 100], f32)
                for q in range(4):
                    csl = slice(q * CCH, (q + 1) * CCH)
                    nsl = slice(q * 25, (q + 1) * 25)
                    nc.sync.dma_start(out=xsl[:, csl],
                                      in_=xsl_d.ap()[:, csl])
                    sq = sqp.tile([125, CCH], f32, tag="sq")
                    nc.scalar.activation(
                        out=sq[:, :], in_=xsl[:, csl],
                        func=mybir.ActivationFunctionType.Square)
                    nc.vector.tensor_reduce(
                        out=ns[:, nsl],
                        in_=sq[:, :].rearrange("p (r d) -> p r d", d=D),
                        axis=mybir.AxisListType.X,
                        op=mybir.AluOpType.add,
                    )
                    nc.scalar.activation(
                        out=nrm[:, nsl], in_=ns[:, nsl],
                        func=mybir.ActivationFunctionType.Sqrt)
                    nc.vector.reciprocal(out=rns[:, nsl], in_=nrm[:, nsl])
                    nc.vector.tensor_mul(
                        out=ntile[:, csl].rearrange("p (r d) -> p r d", d=D),
                        in0=xsl[:, csl].rearrange("p (r d) -> p r d", d=D),
                        in1=rns[:, nsl].unsqueeze(-1).to_broadcast(
                            [125, 25, D]),
                    )
                    # quarter AllGather: output is table bank q
                    agin = dp.tile([QSL, D], bf16, name=f"agin{q}")
                    htab = dp.tile([BANK, D], bf16, name=f"htab{q}",
                                   addr_space="Shared")
                    nc.sync.dma_start(
                        out=agin[:, :].rearrange("(p r) d -> p (r d)", p=125),
                        in_=ntile[:, csl],
                    )
                    nc.gpsimd.collective_compute(
                        "AllGather",
                        mybir.AluOpType.bypass,
                        replica_groups=[list(range(CORES))],
                        ins=[agin.opt()],
                        outs=[htab.opt()],
                    )
                    banks.append(htab)

            # ---- main loop: gathers on 4 queues, DVE dot per call ----
            with tc.tile_pool(name="ga", bufs=5) as ga, \
                 tc.tile_pool(name="gb", bufs=5) as gb:
                qn = 0
                icol_off = 0
                scol_off = 0
                for g in GROUP_ORDER:
                    ba, bb = g // NBANK, g % NBANK
                    for cap in _call_caps(caps[g]):
                        cc = cap // 128   # gathered row-columns this call
                        ic = cap // 16    # index columns this call
                        xs_t = ga.tile([128, (GCALL // 128) * D], bf16,
                                       tag="A")
                        xd_t = gb.tile([128, (GCALL // 128) * D], bf16,
                                       tag="B")
                        nc.gpsimd.dma_gather(
                            out_ap=xs_t[:, :cc * D].rearrange(
                                "p (c d) -> p c d", d=D),
                            in_ap=banks[ba][:, :],
                            idxs_ap=sidx[:, icol_off:icol_off + ic],
                            num_idxs=cap, num_idxs_reg=cap, elem_size=D,
                            single_packet=False, queue_num=qn % 4,
                        )
                        qn += 1
                        nc.gpsimd.dma_gather(
                            out_ap=xd_t[:, :cc * D].rearrange(
                                "p (c d) -> p c d", d=D),
                            in_ap=banks[bb][:, :],
                            idxs_ap=didx[:, icol_off:icol_off + ic],
                            num_idxs=cap, num_idxs_reg=cap, elem_size=D,
                            single_packet=False, queue_num=qn % 4,
                        )
                        qn += 1
                        nc.vector.tensor_mul(out=xs_t[:, :cc * D],
                                             in0=xs_t[:, :cc * D],
                                             in1=xd_t[:, :cc * D])
                        nc.vector.tensor_reduce(
                            out=score[:, scol_off:scol_off + cc],
                            in_=xs_t[:, :cc * D].rearrange(
                                "p (c d) -> p c d", d=D),
                            axis=mybir.AxisListType.X,
                            op=mybir.AluOpType.add,
                        )
                        icol_off += ic
                        scol_off += cc

                nc.sync.dma_start(out=out_d.ap(), in_=score[:, :])

    nc.compile()
    return nc


def _node_map(n):
    """node id -> (bank, bank-local index) for the quarter-AllGather layout.

    AG chunk q gathers quarter q (3125 rows) of every core's slice; core
    r's quarter lands at rows [r*3125, (r+1)*3125) of htab_q = bank q.
    """
    r = n // SLICE
    rem = n - r * SLICE
    q = rem // QSL
    w = rem - q * QSL
    return q, r * QSL + w


def _wrap_idx(flat):
    """[cap] int16 -> [128, cap//16] in dma_gather's 16-partition wrap."""
    blk = flat.reshape(-1, 16).T  # index i at [i%16, i//16]
    return np.tile(blk, (8, 1))


def _group_edges(src_l, dst_l):
    """Group one core's edges by (src_bank, dst_bank); sort by src id."""
    sb, sl = _node_map(src_l)
    db, dl = _node_map(dst_l)
    key = sb * NBANK + db
    order = np.argsort(key, kind="stable")
    sizes = np.bincount(key, minlength=NGRP)
    groups = {}
    off = 0
    for g in range(NGRP):
        ids = order[off:off + sizes[g]]
        off += sizes[g]
        # ascending src addresses give the src-side gather descriptors
        # HBM locality (the dst side stays random)
        ids = ids[np.argsort(sl[ids], kind="stable")]
        groups[g] = (ids, sl[ids], dl[ids])
    return groups


def _pack_core(groups, caps):
    """Build idx tilings + inverse edge map for one core."""
    icols_total = sum(caps) // 16
    sidx = np.zeros((128, icols_total), dtype=np.int16)
    didx = np.zeros((128, icols_total), dtype=np.int16)
    rows = np.empty(EPC, dtype=np.int64)
    cols = np.empty(EPC, dtype=np.int64)
    icol_off = 0
    scol_off = 0
    for g in GROUP_ORDER:
        ids, sl_g, dl_g = groups[g]
        cap_g = caps[g]
        s_pad = np.zeros(cap_g, dtype=np.int16)
        d_pad = np.zeros(cap_g, dtype=np.int16)
        s_pad[:ids.size] = sl_g
        d_pad[:ids.size] = dl_g
        pos = 0
        for cap in _call_caps(cap_g):
            ic = cap // 16
            seg = slice(pos, pos + cap)
            sidx[:, icol_off:icol_off + ic] = _wrap_idx(s_pad[seg])
            didx[:, icol_off:icol_off + ic] = _wrap_idx(d_pad[seg])
            icol_off += ic
            pos += cap
        j = np.arange(ids.size)
        rows[ids] = j % 128
        cols[ids] = scol_off + j // 128
        scol_off += cap_g // 128
    return sidx, didx, rows, cols


def kernel(x, src, dst):
    global LAST_RESULTS
    from concourse.bass_utils import run_bass_kernel_spmd

    x32 = np.ascontiguousarray(np.asarray(x, dtype=np.float32))
    src_i = np.asarray(src).astype(np.int64)
    dst_i = np.asarray(dst).astype(np.int64)

    core_groups = []
    for i in range(CORES):
        core_groups.append(_group_edges(
            src_i[i * EPC:(i + 1) * EPC], dst_i[i * EPC:(i + 1) * EPC]))

    # JIT-exact per-group capacities: max over cores, rounded up to 128
    caps = []
    for g in range(NGRP):
        m = max(cg[g][0].size for cg in core_groups)
        caps.append(((m + 127) // 128) * 128)
    caps = tuple(caps)

    if caps not in _CACHE:
        _CACHE[caps] = _build(caps)
    nc = _CACHE[caps]

    in_maps = []
    inv = []
    for i in range(CORES):
        sidx, didx, rows, cols = _pack_core(core_groups[i], caps)
        inv.append((rows, cols))
        xs = x32[i * SLICE:(i + 1) * SLICE]
        # [12500,128] -> [4,125,25,128] -> [125, 4,25,128] -> [125, 100*128]
        xsl = np.ascontiguousarray(
            xs.reshape(4, 125, 25, D).transpose(1, 0, 2, 3).reshape(
                125, 100 * D))
        in_maps.append({
            "xsl": xsl,
            "src_idx": np.ascontiguousarray(sidx),
            "dst_idx": np.ascontiguousarray(didx),
        })

    res = run_bass_kernel_spmd(nc, in_maps, core_ids=list(range(CORES)),
                               **RUN_KWARGS)
    LAST_RESULTS = res

    out = np.empty(E, dtype=np.float32)
    for i in range(CORES):
        tilev = np.asarray(res.results[i]["out"])
        rows, cols = inv[i]
        out[i * EPC:(i + 1) * EPC] = tilev[rows, cols]
    return out.reshape(E, 1)
